# revision 31
# baseline (speedup 1.0000x reference)
"""Trainium2 Bass kernel for weighted-CE + structural-penalty loss.

Full inputs -> data-parallel shard over batch across 8 NeuronCores ->
per-core Bass kernel computes small fp32 partial sums -> host combines
in float64.

CE trick: the CE term is a plain sum over positions, so it is invariant
under any position permutation. The host sorts each core's positions by
target class into 8 fixed-size bands (PADLEN each, zero-padded), and
rotates the class axis within each band so the target class lands in
class-slot 0. Consequences exploited on device:
 - the gather of the target logit is the contiguous class-0 slice;
 - the per-position CE weight w[t] is CONSTANT per 16-partition band,
   so both CE dot products become matmuls with lhsT = w column:
     g  = sum_p w_p * sum_j x0[p,j]      (pads have x=0 -> contribute 0)
     wl = sum_p w_p * sum_j lse[p,j]     (pads all share one lse value,
                                          host subtracts w.npads*lse_pad)
 - x ships as fp8e4m3 (halves DMA; CE error ~1e-4 relative).

Device per core (F=2096 positions per partition, chunks [128, 492*4]):
 - exp on ACT (fp8 in, fp16 out); se = sum_c exp via identity-lhsT
   matmuls accumulating in PSUM (PE); lse = Ln(se) on ACT.
 - penalty (original order, half-rows on 128 partitions): negated
   cumsum scan (rp-lp), row max (=-min P) via pool, pair terms via
   shifted TT products reduced by ones-matmuls; host chains the row
   halves and adds the one clamped boundary term.
"""

import numpy as np

import concourse.bass as bass
import concourse.mybir as mybir
import concourse.tile as tile
from concourse import bacc
from concourse.bass_utils import run_bass_kernel_spmd

B, S, C = 512, 4096, 8
PENALTY_WEIGHT = 0.1
NCORES = 8
RB = B // NCORES          # batch rows per core
N = RB * S                # real positions per core (262144)
P = 128                   # SBUF partitions
F = 2096                  # padded positions per partition
PADLEN = F * P // C       # positions per class band (33536)
NPAD = F * P              # padded positions per core
CHUNKS = [64, 508, 508, 508, 508]   # position chunks (sum = F)
SH = S // 2               # penalty half-row length
HALO = 3
SW = SH + HALO

F32 = mybir.dt.float32
F16 = mybir.dt.float16
F8 = mybir.dt.float8e4
OP = mybir.AluOpType
AF = mybir.ActivationFunctionType

# [1, x] reduction regions inside the PSUM "red" tile (4 banks).
# bank0: g main [0:508], bank1: wl main [512:1020],
# bank2: pz [1024:1536], bank3: g0 [1536:1600] + wl0 [1600:1664]
RED_G, RED_WL, RED_PZ, RED_G0, RED_WL0 = 0, 512, 1024, 1536, 1600
RED_N = 1664
WIN = 512                 # pz ones-matmul window width


def _patch_act_tables():
    """Prefer the single table set containing Exp+Ln+Copy so the kernel
    pays one ACT_TABLE_LOAD instead of alternating per chunk."""
    import concourse.hw_specs as hw_specs
    if getattr(hw_specs, "_loss_kernel_tables_patched", False):
        return
    orig = hw_specs.get_activation_tables

    def patched(arch):
        t = orig(arch)
        pref = "natural_log_exp_and_others"
        if pref not in t:
            return t
        return {k: (v if k == pref else set()) for k, v in t.items()}

    hw_specs.get_activation_tables = patched
    bacc.get_activation_tables = patched
    hw_specs._loss_kernel_tables_patched = True


def build_program(compile=True):
    _patch_act_tables()
    nc = bacc.Bacc("TRN2", target_bir_lowering=False, debug=False)

    x_d = nc.dram_tensor("x", [P, F * C], F8, kind="ExternalInput").ap()
    si_d = nc.dram_tensor("si", [P, SW + P + 1], F16, kind="ExternalInput").ap()
    red_d = nc.dram_tensor("red", [1, RED_N], F32, kind="ExternalOutput").ap()
    ps_d = nc.dram_tensor("pscan", [P, 2], F32, kind="ExternalOutput").ap()
    lp_d = nc.dram_tensor("lsepad", [1, 1], F16, kind="ExternalOutput").ap()

    nch = len(CHUNKS)
    off = [int(x) for x in np.cumsum([0] + CHUNKS)]

    with tile.TileContext(nc) as tc:
        with (
            tc.tile_pool(name="xb", bufs=1) as xb,
            tc.tile_pool(name="eb", bufs=3) as eb,
            tc.tile_pool(name="stat", bufs=1) as stat,
            tc.tile_pool(name="pen", bufs=1) as pen,
            tc.tile_pool(name="psum", bufs=1, space="PSUM") as psum,
        ):
            si_t = pen.tile([P, SW + P + 1], F16)
            s_t = si_t[:, 0:SW]
            ident = si_t[:, SW:SW + P]
            wcol = si_t[:, SW + P:SW + P + 1]
            ones = stat.tile([P, 1], F16)
            nc.vector.memset(ones, 1.0)
            x_big = xb.tile([P, F * C], F8)

            # input DMA doorbells all issued up front, in arrival-need order
            def dma_x(k):
                nc.sync.dma_start(out=x_big[:, off[k] * C:off[k + 1] * C],
                                  in_=x_d[:, off[k] * C:off[k + 1] * C])
            dma_x(0)
            dma_x(1)
            nc.sync.dma_start(out=si_t, in_=si_d)
            for k in range(2, nch):
                dma_x(k)

            lse = stat.tile([P, F], F16)
            # PSUM: 4 banks for se (reused by chunk 4) + 4 for reductions
            se_ps = [psum.tile([P, 512], F32, name=f"se{k}")
                     for k in range(4)]
            red = psum.tile([1, RED_N], F32, name="red")
            started = set()

            def acc_mm(key, out, lhsT, rhs, last):
                st = key not in started
                started.add(key)
                nc.tensor.matmul(out, lhsT=lhsT, rhs=rhs, start=st, stop=last)

            # ---------------- penalty (starts as soon as si lands) ----
            lp_t = pen.tile([P, SH], F16)
            rp_t = pen.tile([P, SW], F16)
            e_t = pen.tile([P, SW - 1], F16)
            nc.vector.tensor_scalar(out=lp_t, in0=s_t[:, 0:SH], scalar1=1.0,
                                    scalar2=None, op0=OP.is_equal)
            nc.vector.tensor_scalar(out=rp_t, in0=s_t, scalar1=2.0,
                                    scalar2=None, op0=OP.is_equal)
            nc.vector.tensor_scalar(out=e_t, in0=s_t[:, 0:SW - 1], scalar1=3.0,
                                    scalar2=None, op0=OP.is_equal)

            # negated cumsum: p_t = running(rp - lp) = -P ; fp16 is exact
            # for integer values up to +-2048.
            p_t = pen.tile([P, SH], F16)
            nc.vector.tensor_tensor_scan(out=p_t, data0=rp_t[:, 0:SH],
                                         data1=lp_t, initial=0.0,
                                         op0=OP.add, op1=OP.subtract)
            pscan = stat.tile([P, 2], F32)
            nc.vector.tensor_copy(out=pscan[:, 0:1], in_=p_t[:, SH - 1:SH])
            nc.vector.pool(out=pscan[:, 1:2],
                           in_=p_t.rearrange("p (a b) -> p a b", a=1),
                           func=mybir.PoolFunctionType.max)
            nc.gpsimd.dma_start(out=ps_d, in_=pscan)

            # pair terms: pzv[j] = lp[j]*q[j],
            # q[j] = rp[j+1] + e[j+1]*(1.5*rp[j+2] + 2*e[j+2]*rp[j+3])
            w1a = pen.tile([P, SW - 2], F16)
            w1b = pen.tile([P, SW - 2], F16)
            w5a = pen.tile([P, SW - 2], F16)
            w5b = pen.tile([P, SW - 2], F16)
            qa = pen.tile([P, SH], F16)
            qb = pen.tile([P, SH], F16)
            pzv = pen.tile([P, SH], F16)
            nc.vector.tensor_scalar(out=w1a, in0=s_t[:, 1:SW - 1], scalar1=3.0,
                                    scalar2=2.0, op0=OP.is_equal, op1=OP.mult)
            nc.vector.tensor_mul(w1b, w1a, rp_t[:, 2:SW])
            nc.vector.tensor_scalar(out=w5a, in0=s_t[:, 1:SW - 1], scalar1=2.0,
                                    scalar2=1.5, op0=OP.is_equal, op1=OP.mult)
            nc.vector.tensor_add(w5b, w5a, w1b)
            nc.vector.tensor_mul(qa, e_t[:, 1:SH + 1], w5b[:, 1:SH + 1])
            nc.vector.tensor_add(qb, qa, rp_t[:, 1:SH + 1])
            nc.vector.tensor_mul(pzv, lp_t, qb)

            # ---------------- CE chunks ----------------
            for k, cw in enumerate(CHUNKS):
                fl = off[k] * C
                e_x = eb.tile([P, C, 512], F16, tag="e")
                xk = x_big[:, fl:fl + C * cw].rearrange(
                    "p (c w) -> p c w", c=C)
                nc.scalar.activation(e_x[:, :, 0:cw], xk, AF.Exp)
                sp = se_ps[k % 4]
                for c in range(C):
                    nc.tensor.matmul(sp[:, 0:cw], lhsT=ident,
                                     rhs=e_x[:, c, 0:cw],
                                     start=(c == 0), stop=(c == C - 1))
                if k == nch - 1:
                    for w in range(SH // WIN):
                        acc_mm(("pz",), red[:, RED_PZ:RED_PZ + WIN],
                               lhsT=ones, rhs=pzv[:, w * WIN:(w + 1) * WIN],
                               last=(w == SH // WIN - 1))
                ksl = slice(off[k], off[k + 1])
                nc.scalar.activation(lse[:, ksl], sp[:, 0:cw], AF.Ln)
                # CE dot products: lhsT = w column (per-band weights);
                # class-0 block of chunk k sits at x_big[:, fl:fl+cw].
                if k == 0:
                    acc_mm(("g0",), red[:, RED_G0:RED_G0 + cw], lhsT=wcol,
                           rhs=x_big[:, fl:fl + cw], last=True)
                    acc_mm(("wl0",), red[:, RED_WL0:RED_WL0 + cw], lhsT=wcol,
                           rhs=lse[:, ksl], last=True)
                else:
                    acc_mm(("g",), red[:, RED_G:RED_G + cw], lhsT=wcol,
                           rhs=x_big[:, fl:fl + cw], last=(k == nch - 1))
                    acc_mm(("wl",), red[:, RED_WL:RED_WL + cw], lhsT=wcol,
                           rhs=lse[:, ksl], last=(k == nch - 1))

            nc.sync.dma_start(out=lp_d, in_=lse[P - 1:P, F - 1:F])
            red_sb = stat.tile([1, RED_N], F32)
            nc.vector.tensor_copy(out=red_sb[:, 1024:RED_N],
                                  in_=red[:, 1024:RED_N])
            nc.scalar.activation(red_sb[:, 0:1024], red[:, 0:1024], AF.Copy)
            nc.sync.dma_start(out=red_d, in_=red_sb)

    if compile:
        nc.compile()
    return nc


_program = None


def _get_program():
    global _program
    if _program is None:
        _program = build_program()
    return _program


def _pair_boundary(s):
    """The only clamped boundary pair term not covered on device:
    4 * [s[S-3]==1][s[S-2]==3][s[S-1]==2] per row."""
    m = (s[:, -3] == 1) & (s[:, -2] == 3) & (s[:, -1] == 2)
    return 4.0 * float(m.sum())


def combine_partials(results, s_full, nnz, wpad_sums):
    gs = 0.0
    wl = 0.0
    pz = 0.0
    pen = 0.0
    for i, r in enumerate(results):
        red = r["red"].astype(np.float64).ravel()
        gs += red[RED_G:RED_G + 508].sum() + red[RED_G0:RED_G0 + 64].sum()
        wl += red[RED_WL:RED_WL + 508].sum() + red[RED_WL0:RED_WL0 + 64].sum()
        # subtract the pad contributions to wl (all pads share one lse)
        lse_pad = float(np.float64(r["lsepad"].ravel()[0]))
        wl -= lse_pad * wpad_sums[i]
        pz += red[RED_PZ:RED_PZ + WIN].sum()
        sc = r["pscan"].astype(np.float64)
        pf, mp = -sc[:, 0], -sc[:, 1]   # undo the negated scan
        pfa, mpa = pf[0:RB], mp[0:RB]
        pfb, mpb = pf[RB:P], mp[RB:P]
        pft = pfa + pfb
        mpt = np.minimum(mpa, pfa + mpb)
        pen += (pft - 2.0 * np.minimum(0.0, mpt)).sum()
    pen += 2.0 * pz
    pen += _pair_boundary(s_full)
    ce_loss = (wl - gs) / (B * S)
    penalty = pen / nnz
    return np.float32(ce_loss + PENALTY_WEIGHT * penalty)


def make_in_maps(logits, targets, predicted_structures, ce_weights):
    import ml_dtypes
    lg = np.asarray(logits).astype(ml_dtypes.float8_e4m3fn)
    t = np.asarray(targets, dtype=np.int64)
    w16 = np.asarray(ce_weights, dtype=np.float16)
    s = np.ascontiguousarray(
        np.asarray(predicted_structures).reshape(B, S), dtype=np.float16)
    ident = np.eye(P, dtype=np.float16)
    wcol = np.repeat(w16, P // C).reshape(P, 1)
    in_maps = []
    wpad_sums = []
    for core in range(NCORES):
        rows = slice(core * RB, (core + 1) * RB)
        tc = t[rows].ravel()
        cnt = np.bincount(tc, minlength=C)
        assert cnt.max() <= PADLEN, f"class count {cnt.max()} > PADLEN"
        assert cnt[C - 1] < PADLEN, "band 7 has no pad cell"
        perm = np.argsort(tc, kind="stable")
        xs = lg[rows].reshape(N, C)[perm]
        xp = np.zeros((NPAD, C), ml_dtypes.float8_e4m3fn)
        pos = 0
        for c in range(C):
            band = xs[pos:pos + cnt[c]]
            # rotate class axis: target class -> slot 0
            xp[c * PADLEN:c * PADLEN + cnt[c]] = np.concatenate(
                [band[:, c:], band[:, :c]], axis=1)
            pos += cnt[c]
        # sum_p w_p * npads_p  (for the host-side wl pad correction)
        iband = np.arange(P) % (P // C)
        real_p = np.clip(cnt[np.arange(P) // (P // C)] - iband * F, 0, F)
        wpad_sums.append(float(
            (np.float64(wcol.ravel()) * (F - real_p)).sum()))
        # [P, F, C] -> class-blocked per chunk [P, sum_k C*w]
        xp = xp.reshape(P, F, C)
        xcore = np.empty((P, F * C), ml_dtypes.float8_e4m3fn)
        o = 0
        a = 0
        for cw in CHUNKS:
            blk = xp[:, a:a + cw, :].transpose(0, 2, 1)  # [P, C, cw]
            xcore[:, o:o + C * cw] = blk.reshape(P, C * cw)
            o += C * cw
            a += cw
        sc = s[rows]
        s_pack = np.zeros((P, SW), np.float16)
        s_pack[0:RB] = sc[:, 0:SW]
        s_pack[RB:P, 0:SH] = sc[:, SH:S]
        in_maps.append({
            "x": xcore,
            "si": np.concatenate([s_pack, ident, wcol], axis=1),
        })
    return in_maps, wpad_sums


def kernel(logits, targets, predicted_structures, ce_weights):
    in_maps, wpad_sums = make_in_maps(
        logits, targets, predicted_structures, ce_weights)
    t = np.asarray(targets)
    nnz = float(B * S - int((t == 0).sum()))
    s_full = np.asarray(predicted_structures).reshape(B, S)
    nc = _get_program()
    res = run_bass_kernel_spmd(nc, in_maps, core_ids=list(range(NCORES)))
    return combine_partials(res.results, s_full, nnz, wpad_sums)
